# revision 24
# baseline (speedup 1.0000x reference)
"""Trainium2 Bass kernel for nn_FAVORiserBlock (Performer gated transformer block).

Sharding: 8 cores; core c handles batch b=c//2, token-half h=c%2 (1024 of 2048
tokens). The FAVOR+ key-side statistics (global key max, k_sum, ctx) need the
full 2048-token sequence, so each core recomputes the key side for its whole
batch (~8% extra FLOPs) — zero cross-core communication, pure SPMD. The host
rotates each core's sequence so that its own 1024 tokens come first, which
leaves key-side sums/maxes unchanged (order-invariant reductions).

All activations are kept feature-major ([d, tokens], d on partitions) so every
matmul consumes them directly; the host pre-transposes x and post-transposes
the output. Matmuls run as float32r (full PE rate at N>=256, ~1e-4 rel err).
"""
import sys

sys.path.insert(0, "/opt/trn_rl_repo")

from contextlib import ExitStack

import numpy as np

import concourse.bass as bass
import concourse.mybir as mybir
import concourse.tile as tile
from concourse import bacc
from concourse.bass import ts, ds
from concourse.bass_utils import run_bass_kernel_spmd
from concourse.masks import make_identity

F32 = mybir.dt.float32
MMDT = mybir.dt.float32r
AX = mybir.AxisListType
OP = mybir.AluOpType
AF = mybir.ActivationFunctionType

# dims (hardcoded for this problem)
D = 1024          # d_model
DK = D // 128     # 8 feature k-tiles
INNER = 512
H = 8
DH = 64
MF = 266          # FAVOR+ features
MFP = MF + 1      # +1 ones/eps column
TF = 2048         # full sequence (per batch)
TM = 1024         # tokens owned by this core
NTF = TF // 128
NTM = TM // 128
FF = 4096
CH = 256          # phase-1 LayerNorm chunk (tokens)

DN = float(64 ** -0.25)
RATIO = float(266 ** -0.5)
LNRATIO = float(np.log(RATIO))
EPSK = 1e-4
EPSR = RATIO * EPSK
EPSLN = 1e-5
DIAG_SCALE = 0.5 * DN * DN  # multiplies sum(k^2)

N_CORES = 8
BATCH, SEQ = 4, 2048

WEIGHT_SHAPES = dict(
    proj_W=[D, D], Wq=[D, INNER], Wk=[D, INNER], Wv=[D, INNER], Wo=[INNER, D],
    pW1=[D, FF], pW2=[FF, D], Wf1=[D, FF], Wf2=[FF, D],
)
VEC_SHAPES = dict(
    ln_g=D, ln_b=D, a_ln_g=D, a_ln_b=D, f_ln_g=D, f_ln_b=D,
    proj_b=D, bq=INNER, bk=INNER, bv=INNER, bo=D,
    pb1=FF, pb2=D, bf1=FF, bf2=D,
)


def r(ap):
    return ap.bitcast(MMDT)


def build_nc(debug=False):
    nc = bacc.Bacc("TRN2", target_bir_lowering=False, debug=False)

    xT = nc.dram_tensor("xT", [D, TF], MMDT, kind="ExternalInput")
    projTdn = nc.dram_tensor("projTdn", [DH, MF], MMDT, kind="ExternalInput")
    W = {k: nc.dram_tensor(k, v, MMDT, kind="ExternalInput") for k, v in WEIGHT_SHAPES.items()}
    V = {k: nc.dram_tensor(k, [v], F32, kind="ExternalInput") for k, v in VEC_SHAPES.items()}
    outT = nc.dram_tensor("outT", [D, TM], F32, kind="ExternalOutput")
    u_dram = nc.dram_tensor("u_scratch", [D, TM], F32)
    x1_dram = nc.dram_tensor("x1_scratch", [D, TM], MMDT)
    dbg = {}
    if debug:
        dbg["y0"] = nc.dram_tensor("dbg_y0", [128, DK, TM], F32, kind="ExternalOutput")
        dbg["k"] = nc.dram_tensor("dbg_k", [128, 4, TF], F32, kind="ExternalOutput")
        dbg["q"] = nc.dram_tensor("dbg_q", [128, 4, TM], F32, kind="ExternalOutput")
        dbg["vv"] = nc.dram_tensor("dbg_vv", [128, NTF, H, 65], F32, kind="ExternalOutput")
        dbg["o"] = nc.dram_tensor("dbg_o", [128, 4, TM], F32, kind="ExternalOutput")
        dbg["v1"] = nc.dram_tensor("dbg_v1", [128, DK, TM], F32, kind="ExternalOutput")
        dbg["ident"] = nc.dram_tensor("dbg_ident", [128, 128], F32, kind="ExternalOutput")
        dbg["ksum"] = nc.dram_tensor("dbg_ksum", [128, MF], F32, kind="ExternalOutput")
        dbg["gmax"] = nc.dram_tensor("dbg_gmax", [128, 1], F32, kind="ExternalOutput")
        dbg["mkb"] = nc.dram_tensor("dbg_mkb", [128, 1], F32, kind="ExternalOutput")
        dbg["diag"] = nc.dram_tensor("dbg_diag", [128, NTF], F32, kind="ExternalOutput")
        dbg["kp"] = nc.dram_tensor("dbg_kp", [128, 268], F32, kind="ExternalOutput")
        dbg["ctxr"] = nc.dram_tensor("dbg_ctxr", [65, 268], F32, kind="ExternalOutput")
        dbg["ctxsb"] = nc.dram_tensor("dbg_ctxsb", [65, MF], F32, kind="ExternalOutput")
        dbg["qpT"] = nc.dram_tensor("dbg_qpT", [128, 3, TM], F32, kind="ExternalOutput")
        dbg["ctxT"] = nc.dram_tensor("dbg_ctxT", [128, 3, DH], F32, kind="ExternalOutput")

    xT_v = xT.rearrange("(kk p) t -> p kk t", p=128)           # [128, DK, TF]
    projW_v = W["proj_W"].rearrange("(kk p) n -> p kk n", p=128)
    Wq_v = W["Wq"].rearrange("(kk p) n -> p kk n", p=128)
    Wk_v = W["Wk"].rearrange("(kk p) n -> p kk n", p=128)
    Wv_v = W["Wv"].rearrange("(kk p) n -> p kk n", p=128)
    Wo_v = W["Wo"].rearrange("(kk p) n -> p kk n", p=128)      # [128, 4, D]
    pW1_v = W["pW1"].rearrange("(kk p) n -> p kk n", p=128)
    pW2_v = W["pW2"].rearrange("(kk p) n -> p kk n", p=128)    # [128, 32, D]
    Wf1_v = W["Wf1"].rearrange("(kk p) n -> p kk n", p=128)
    Wf2_v = W["Wf2"].rearrange("(kk p) n -> p kk n", p=128)

    with tile.TileContext(nc) as tc, ExitStack() as top:
        const = top.enter_context(tc.tile_pool(name="const", bufs=1))

        # ---- constants ----
        identF = const.tile([128, 128], F32)
        make_identity(nc, identF[:])
        ident = const.tile([128, 128], MMDT)
        nc.gpsimd.dma_start(ident[:], identF[:])     # cast f32 -> f32r
        onesF = const.tile([128, 128], F32)
        nc.vector.memset(onesF[:], 1.0)
        ones128 = const.tile([128, 1], MMDT)
        nc.gpsimd.dma_start(ones128[:], onesF[:, 0:1])
        ones_pair = const.tile([128, 2], MMDT)
        nc.gpsimd.dma_start(ones_pair[:], onesF[:, 0:2])
        projT2 = const.tile([128, MF], MMDT)  # projT duplicated to both halves
        nc.sync.dma_start(projT2[0:DH, :], projTdn[:, :])
        nc.sync.dma_start(projT2[DH:128, :], projTdn[:, :])
        eps1 = const.tile([1, 1], F32)
        nc.vector.memset(eps1[:], EPSLN)

        def vec_tile(name, n):
            t = const.tile([128, n // 128], F32, tag=f"v_{name}")
            nc.sync.dma_start(t[:], V[name].rearrange("(k p) -> p k", p=128))
            return t

        lng, lnb = vec_tile("ln_g", D), vec_tile("ln_b", D)
        alng, alnb = vec_tile("a_ln_g", D), vec_tile("a_ln_b", D)
        flng, flnb = vec_tile("f_ln_g", D), vec_tile("f_ln_b", D)
        projb_t = vec_tile("proj_b", D)
        bq_t, bk_t = vec_tile("bq", INNER), vec_tile("bk", INNER)
        bo_t, pb2_t, bf2_t = vec_tile("bo", D), vec_tile("pb2", D), vec_tile("bf2", D)
        pb1_t, bf1_t = vec_tile("pb1", FF), vec_tile("bf1", FF)
        bv_row = const.tile([1, INNER], F32)
        nc.sync.dma_start(bv_row[:], V["bv"].rearrange("(a n) -> a n", a=1))
        bv_b = const.tile([128, INNER], F32)
        nc.gpsimd.partition_broadcast(bv_b[:], bv_row[:])

        ylife = top.enter_context(tc.tile_pool(name="ylife", bufs=1))
        y0buf = ylife.tile([128, DK, TM], MMDT, tag="y0")  # my-half y0; becomes v1

        # =============================================================
        # LayerNorm helper (feature-major): stats via ones-matmuls
        # =============================================================
        def layernorm(src_fn, dst_fn, width, g, b, pools):
            strm, st, psums = pools
            psum_s = psums.tile([1, width], F32, tag="ln_s")
            psum_q = psums.tile([1, width], F32, tag="ln_q")
            for kk in range(DK):
                sq = strm.tile([128, width], MMDT, tag="sq")
                nc.vector.tensor_mul(sq[:], src_fn(kk), src_fn(kk))
                nc.tensor.matmul(psum_s[:], r(ones128[:]), r(src_fn(kk)),
                                 start=(kk == 0), stop=(kk == DK - 1))
                nc.tensor.matmul(psum_q[:], r(ones128[:]), r(sq[:]),
                                 start=(kk == 0), stop=(kk == DK - 1))
            mu = st.tile([1, width], F32, tag="mu")
            nc.vector.tensor_scalar_mul(mu[:], psum_s[:], 1.0 / D)
            mu2 = st.tile([1, width], F32, tag="tA")
            nc.vector.tensor_mul(mu2[:], mu[:], mu[:])
            var = st.tile([1, width], F32, tag="tB")
            nc.vector.scalar_tensor_tensor(var[:], psum_q[:], 1.0 / D, mu2[:],
                                           op0=OP.mult, op1=OP.subtract)
            std = st.tile([1, width], F32, tag="tA")
            nc.scalar.activation(std[:], var[:], AF.Sqrt, bias=eps1[:], scale=1.0)
            s = st.tile([1, width], F32, tag="tB")
            nc.vector.reciprocal(s[:], std[:])
            nms = st.tile([1, width], F32, tag="tA")
            nc.vector.scalar_tensor_tensor(nms[:], mu[:], -1.0, s[:],
                                           op0=OP.mult, op1=OP.mult)
            A_b = st.tile([128, width], F32, tag="A_b")
            B_b = st.tile([128, width], F32, tag="B_b")
            nc.gpsimd.partition_broadcast(A_b[:], s[:])
            nc.gpsimd.partition_broadcast(B_b[:], nms[:])
            for kk in range(DK):
                t1 = strm.tile([128, width], F32, tag="t1")
                nc.vector.tensor_mul(t1[:], src_fn(kk), A_b[:])
                nc.vector.tensor_add(t1[:], t1[:], B_b[:])
                nc.vector.tensor_scalar(dst_fn(kk), t1[:], g[:, kk:kk + 1],
                                        b[:, kk:kk + 1], op0=OP.mult, op1=OP.add)

        with ExitStack() as ph12:
            pA = ph12.enter_context(tc.tile_pool(name="pA", bufs=1))
            kfm = pA.tile([128, 4, TF], MMDT, tag="kfm")        # k features [512, TF]
            qfm = pA.tile([128, 4, TM], MMDT, tag="qfm")
            vvbuf = pA.tile([128, NTF, H, 65], MMDT, tag="vv")  # token-major v + ones
            _oa = ones128[:]
            _ones_b = bass.AP(tensor=_oa.tensor, offset=_oa.offset,
                              ap=[list(_oa.ap[0]), [0, NTF], [0, H], [0, 1]])
            nc.vector.tensor_copy(vvbuf[:, :, :, 64:65], _ones_b)

            # =========================================================
            # Phase 1: LN1 -> LN2 -> Q/K/V projections, per 512-token tile
            # =========================================================
            with ExitStack() as ph1:
                strm = ph1.enter_context(tc.tile_pool(name="p1s", bufs=2))
                one1 = ph1.enter_context(tc.tile_pool(name="p1o", bufs=1))
                st = ph1.enter_context(tc.tile_pool(name="p1st", bufs=1))
                psums = ph1.enter_context(tc.tile_pool(name="p1ps", bufs=2, space="PSUM"))
                lnpools = (strm, st, psums)

                for half in range(2):
                    for tq in range(2):
                        tg = half * TM + tq * 512   # global token offset
                        y1q = one1.tile([128, DK, 512], MMDT, tag="y1q")
                        for chi in range(2):
                            t0 = tg + chi * CH
                            xin = one1.tile([128, DK, CH], MMDT, tag="xin")
                            nc.sync.dma_start(xin[:], xT_v[:, :, ds(t0, CH)])
                            if half == 0:
                                loc = tq * 512 + chi * CH
                                y0dst = lambda kk, lo=loc: y0buf[:, kk, ds(lo, CH)]
                            else:
                                y0t = one1.tile([128, DK, CH], MMDT, tag="y0t")
                                y0dst = lambda kk, t=y0t: t[:, kk, :]
                            layernorm(lambda kk, x=xin: x[:, kk, :], y0dst, CH,
                                      lng, lnb, lnpools)
                            layernorm(y0dst,
                                      lambda kk, c=chi, y=y1q: y[:, kk, ds(c * CH, CH)],
                                      CH, alng, alnb, lnpools)

                        # feature-major K (and Q for my half) projections
                        plist = [(Wk_v, bk_t, kfm, tg)]
                        if half == 0:
                            plist.append((Wq_v, bq_t, qfm, tq * 512))
                        for (wv_, bias_t, dstbuf, dsto) in plist:
                            for m in range(4):
                                wt = strm.tile([128, DK, 128], MMDT, tag="wkq")
                                nc.sync.dma_start(wt[:], wv_[:, :, ts(m, 128)])
                                ps = psums.tile([128, 512], F32, tag="mm")
                                for kk in range(DK):
                                    nc.tensor.matmul(ps[:], r(wt[:, kk, :]),
                                                     r(y1q[:, kk, :]),
                                                     start=(kk == 0), stop=(kk == DK - 1))
                                nc.vector.tensor_scalar(
                                    dstbuf[:, m, ds(dsto, 512)], ps[:],
                                    bias_t[:, m:m + 1], None, op0=OP.add)

                        # token-major V (bias broadcast along free dim)
                        wvt = one1.tile([128, DK, INNER], MMDT, tag="wv")
                        nc.sync.dma_start(wvt[:], Wv_v[:, :, :])
                        for nt in range(4):
                            ps = psums.tile([128, INNER], F32, tag="mm")
                            for kk in range(DK):
                                nc.tensor.matmul(ps[:], r(y1q[:, kk, ts(nt, 128)]),
                                                 r(wvt[:, kk, :]),
                                                 start=(kk == 0), stop=(kk == DK - 1))
                            gnt = half * NTM + tq * 4 + nt
                            nc.vector.tensor_add(
                                vvbuf[:, gnt, :, 0:64],
                                ps[:].rearrange("p (h d) -> p h d", h=H),
                                bv_b[:].rearrange("p (h d) -> p h d", h=H))

            if debug:
                nc.sync.dma_start(dbg["y0"][:], y0buf[:].bitcast(F32))
                nc.sync.dma_start(dbg["k"][:], kfm[:].bitcast(F32))
                nc.sync.dma_start(dbg["q"][:], qfm[:].bitcast(F32))
                nc.sync.dma_start(dbg["vv"][:], vvbuf[:].bitcast(F32))
                nc.sync.dma_start(dbg["ident"][:], ident[:].bitcast(F32))

            # =========================================================
            # Phase 2a: u = y0 @ proj_W + proj_b -> spilled to DRAM
            # =========================================================
            with ExitStack() as ph2:
                wstrm = ph2.enter_context(tc.tile_pool(name="p2w", bufs=2))
                apool = ph2.enter_context(tc.tile_pool(name="p2a", bufs=2))
                abig = ph2.enter_context(tc.tile_pool(name="p2b", bufs=1))
                psums = ph2.enter_context(tc.tile_pool(name="p2ps", bufs=2, space="PSUM"))
                psacc = ph2.enter_context(tc.tile_pool(name="p2pa", bufs=1, space="PSUM"))

                for m in range(DK):
                    wt = wstrm.tile([128, DK, 128], MMDT, tag="wu")
                    nc.sync.dma_start(wt[:], projW_v[:, :, ts(m, 128)])
                    for t2 in range(2):
                        ps = psums.tile([128, 512], F32, tag="mm")
                        for kk in range(DK):
                            nc.tensor.matmul(ps[:], r(wt[:, kk, :]),
                                             r(y0buf[:, kk, ds(t2 * 512, 512)]),
                                             start=(kk == 0), stop=(kk == DK - 1))
                        ut = wstrm.tile([128, 512], F32, tag="uout")
                        nc.vector.tensor_scalar(ut[:], ps[:], projb_t[:, m:m + 1],
                                                None, op0=OP.add)
                        nc.sync.dma_start(u_dram[ts(m, 128), ds(t2 * 512, 512)], ut[:])

                # =====================================================
                # Phase 2b: FAVOR+ attention, head pairs
                # =====================================================
                obuf = abig.tile([128, 4, TM], MMDT, tag="obuf")
                for hp in range(4):
                    ksqt = abig.tile([128, TF], MMDT, tag="ksq")
                    nc.vector.tensor_mul(ksqt[:], kfm[:, hp, :], kfm[:, hp, :])
                    qsqt = abig.tile([128, TM], MMDT, tag="qsq")
                    nc.vector.tensor_mul(qsqt[:], qfm[:, hp, :], qfm[:, hp, :])
                    for sub in range(2):
                        h = 2 * hp + sub
                        lo = 64 * sub
                        hs = slice(lo, lo + 64)

                        # ---- key side: pass A (global dd max, diag) ----
                        diag_k = apool.tile([128, NTF], F32, tag="dgk")
                        gmax = apool.tile([128, 1], F32, tag="gmax")
                        for nt in range(NTF):
                            psd = psums.tile([128, 272], F32, tag="dd")
                            nc.tensor.matmul(psd[:, 0:MF],
                                             r(kfm[hs, hp, ts(nt, 128)]),
                                             r(projT2[hs, :]), start=True, stop=True)
                            nc.tensor.matmul(psd[:, 268:270],
                                             r(ksqt[hs, ts(nt, 128)]),
                                             ones_pair[hs, :], start=True, stop=True)
                            nc.vector.tensor_scalar_mul(diag_k[:, nt:nt + 1],
                                                        psd[:, 268:269], DIAG_SCALE)
                            mx = apool.tile([128, 1], F32, tag="mx")
                            nc.vector.tensor_reduce(mx[:], psd[:, 0:MF], axis=AX.X,
                                                    op=OP.max)
                            if nt == 0:
                                nc.vector.tensor_copy(gmax[:], mx[:])
                            else:
                                nc.vector.tensor_max(gmax[:], gmax[:], mx[:])
                        ptr = psums.tile([128, 512], F32, tag="big")
                        nc.tensor.transpose(ptr[0:1, 0:128], gmax[:], identF[:])
                        mks = apool.tile([1, 1], F32, tag="mks")
                        nc.vector.tensor_reduce(mks[:], ptr[0:1, 0:128], axis=AX.X,
                                                op=OP.max)
                        mkb = apool.tile([128, 1], F32, tag="mkb")
                        nc.gpsimd.partition_broadcast(mkb[:], mks[:])

                        if debug and h == 0:
                            nc.sync.dma_start(dbg["gmax"][:], gmax[:])
                            nc.sync.dma_start(dbg["mkb"][:], mkb[:])
                            nc.sync.dma_start(dbg["diag"][:], diag_k[:])
                        # ---- key side: pass B (kp, ctx, k_sum) ----
                        pctx = psacc.tile([65, 268], F32, tag="ctx")
                        for nt in range(NTF):
                            psd = psums.tile([128, 272], F32, tag="dd")
                            nc.tensor.matmul(psd[:, 0:MF],
                                             r(kfm[hs, hp, ts(nt, 128)]),
                                             r(projT2[hs, :]), start=True, stop=True)
                            biask = apool.tile([128, 1], F32, tag="bk")
                            nc.vector.scalar_tensor_tensor(
                                biask[:], diag_k[:, nt:nt + 1], -1.0, mkb[:],
                                op0=OP.mult, op1=OP.subtract)
                            nc.vector.tensor_scalar_add(biask[:], biask[:], LNRATIO)
                            kp = apool.tile([128, 268], MMDT, tag="kp")
                            nc.scalar.activation(kp[:, 0:MF], psd[:, 0:MF], AF.Exp,
                                                 bias=biask[:], scale=1.0)
                            _ka = ones128[:]
                            nc.vector.tensor_copy(
                                kp[:, MF:268],
                                bass.AP(tensor=_ka.tensor, offset=_ka.offset,
                                        ap=[list(_ka.ap[0]), [0, 2]]))
                            if debug and h == 0 and nt == 0:
                                nc.sync.dma_start(dbg["kp"][:], kp[:].bitcast(F32))
                            nc.tensor.matmul(pctx[:], r(vvbuf[:, nt, h, :]), r(kp[:]),
                                             start=(nt == 0), stop=(nt == NTF - 1))
                        # fold eps column: ctx_sb = pctx[:, :MF] + EPSR*pctx[:, MF]
                        # (stage psum->sbuf first: one DVE op cannot read two
                        #  PSUM operands)
                        ctx_raw = apool.tile([65, 268], F32, tag="ctxraw")
                        nc.vector.tensor_copy(ctx_raw[:], pctx[:])
                        ctx_sb = apool.tile([65, MF], F32, tag="ctxsb")
                        nc.vector.scalar_tensor_tensor(
                            ctx_sb[:], ctx_raw[:, MF:MFP].broadcast_to((65, MF)), EPSR,
                            ctx_raw[:, 0:MF], op0=OP.mult, op1=OP.add)
                        if debug and h == 0:
                            nc.sync.dma_start(dbg["ctxr"][:], ctx_raw[:])
                            nc.sync.dma_start(dbg["ctxsb"][:], ctx_sb[:].bitcast(F32))
                        # partition_broadcast on HW reads physical partition 0
                        # regardless of AP base -> stage row 64 to partition 0
                        ksrow = apool.tile([1, MF], F32, tag="ksrow")
                        nc.sync.dma_start(ksrow[:], ctx_sb[64:65, :].bitcast(F32))
                        ksum_b = apool.tile([128, MF], F32, tag="ksb")
                        nc.gpsimd.partition_broadcast(ksum_b[:], ksrow[:])
                        ctxsum = apool.tile([65, 1], F32, tag="ctxsum")
                        with nc.allow_low_precision(reason="f32r ctxsum; fp32-internal DVE reduce"):
                            nc.vector.tensor_reduce(ctxsum[:], ctx_sb[:],
                                                    axis=AX.X, op=OP.add)
                        srow = apool.tile([1, 1], F32, tag="srow")
                        nc.sync.dma_start(srow[:], ctxsum[64:65, 0:1])
                        Sb = apool.tile([128, 1], F32, tag="Sb")
                        nc.gpsimd.partition_broadcast(Sb[:], srow[:])
                        # ctxT: [m-chunk, c, dh] + ctxsum row at m=266 (chunk2, 10)
                        ctxT = abig.tile([128, 3, DH], MMDT, tag="ctxT")
                        for c in range(3):
                            w = min(128, MF - c * 128)
                            ptt = psums.tile([128, 512], F32, tag="big")
                            nc.tensor.transpose(ptt[0:w, 0:DH],
                                                ctx_sb[0:64, ds(c * 128, w)],
                                                identF[0:64, 0:64])
                            nc.vector.tensor_copy(ctxT[0:w, c, :], ptt[0:w, 0:DH])
                        ptt2 = psums.tile([128, 512], F32, tag="big")
                        nc.tensor.transpose(ptt2[0:1, 0:DH], ctxsum[0:64, :],
                                            identF[0:64, 0:64])
                        csrow = apool.tile([1, DH], F32, tag="csrow")
                        nc.vector.tensor_copy(csrow[:], ptt2[0:1, 0:DH])
                        nc.gpsimd.dma_start(ctxT[10:11, 2, :], csrow[:])  # cast f32->f32r

                        if debug and h == 0:
                            nc.sync.dma_start(dbg["ksum"][:], ksum_b[:])
                        # ---- query side ----
                        qpT = abig.tile([128, 3, TM], MMDT, tag="qpT")
                        for nt in range(NTM):
                            psd = psums.tile([128, 272], F32, tag="dd")
                            nc.tensor.matmul(psd[:, 0:MF],
                                             r(qfm[hs, hp, ts(nt, 128)]),
                                             r(projT2[hs, :]), start=True, stop=True)
                            nc.tensor.matmul(psd[:, 268:270],
                                             r(qsqt[hs, ts(nt, 128)]),
                                             ones_pair[hs, :], start=True, stop=True)
                            mrow = apool.tile([128, 1], F32, tag="mrow")
                            nc.vector.tensor_reduce(mrow[:], psd[:, 0:MF], axis=AX.X,
                                                    op=OP.max)
                            biasq = apool.tile([128, 1], F32, tag="bq")
                            nc.vector.scalar_tensor_tensor(
                                biasq[:], psd[:, 268:269], -DIAG_SCALE, mrow[:],
                                op0=OP.mult, op1=OP.subtract)
                            nc.vector.tensor_scalar_add(biasq[:], biasq[:], LNRATIO)
                            qp = apool.tile([128, MF], MMDT, tag="qp")
                            nc.scalar.activation(qp[:], psd[:, 0:MF], AF.Exp,
                                                 bias=biasq[:], scale=1.0)
                            den = apool.tile([128, 1], F32, tag="den")
                            trash = apool.tile([128, MF], F32, tag="trash")
                            nc.vector.scalar_tensor_tensor(
                                trash[:], qp[:], 1.0, ksum_b[:], op0=OP.bypass,
                                op1=OP.mult, accum_out=den[:])
                            den2 = apool.tile([128, 1], F32, tag="den2")
                            nc.vector.scalar_tensor_tensor(den2[:], Sb[:], EPSR,
                                                           den[:], op0=OP.mult,
                                                           op1=OP.add)
                            dinv = apool.tile([128, 1], F32, tag="dinv")
                            nc.vector.reciprocal(dinv[:], den2[:])
                            qps = apool.tile([128, MFP], MMDT, tag="qps")
                            nc.vector.tensor_scalar(qps[:, 0:MF], qp[:], dinv[:],
                                                    None, op0=OP.mult)
                            nc.vector.tensor_scalar(qps[:, MF:MFP], dinv[:], EPSR,
                                                    None, op0=OP.mult)
                            for c in range(3):
                                w = 128 if c < 2 else MFP - 256
                                ptq = psums.tile([128, 512], F32, tag="big")
                                nc.tensor.transpose(r(ptq[0:w, 0:128]),
                                                    qps[:, ds(c * 128, w)], ident[:])
                                nc.vector.tensor_copy(qpT[0:w, c, ts(nt, 128)],
                                                      ptq[0:w, 0:128])
                        if debug and h == 0:
                            nc.sync.dma_start(dbg["qpT"][:], qpT[:].bitcast(F32))
                            nc.sync.dma_start(dbg["ctxT"][:], ctxT[:].bitcast(F32))
                        # ---- o_h = qps @ ctx ----
                        for t2 in range(2):
                            po = psums.tile([128, 512], F32, tag="big")
                            for c in range(3):
                                w = 128 if c < 2 else 11
                                nc.tensor.matmul(po[0:64, :], ctxT[0:w, c, :],
                                                 qpT[0:w, c, ds(t2 * 512, 512)],
                                                 start=(c == 0), stop=(c == 2))
                            if sub == 0:
                                nc.vector.tensor_copy(
                                    obuf[0:64, hp, ds(t2 * 512, 512)], po[0:64, :])
                            else:
                                otmp = apool.tile([64, 512], MMDT, tag="otmp")
                                nc.vector.tensor_copy(otmp[:], po[0:64, :])
                                nc.sync.dma_start(
                                    obuf[64:128, hp, ds(t2 * 512, 512)], otmp[:])

                # =====================================================
                # Phase 2c: v1 = y0 + o @ Wo + bo (in-place into y0buf)
                # =====================================================
                for m in range(DK):
                    wt = wstrm.tile([128, 4, 128], MMDT, tag="wo")
                    nc.sync.dma_start(wt[:], Wo_v[:, :, ts(m, 128)])
                    for t2 in range(2):
                        ps = psums.tile([128, 512], F32, tag="mm")
                        for kk in range(4):
                            nc.tensor.matmul(ps[:], r(wt[:, kk, :]),
                                             r(obuf[:, kk, ds(t2 * 512, 512)]),
                                             start=(kk == 0), stop=(kk == 3))
                        nc.vector.scalar_tensor_tensor(
                            y0buf[:, m, ds(t2 * 512, 512)], ps[:], bo_t[:, m:m + 1],
                            y0buf[:, m, ds(t2 * 512, 512)], op0=OP.add, op1=OP.add)

            if debug:
                nc.sync.dma_start(dbg["o"][:], obuf[:].bitcast(F32))

        if debug:
            nc.sync.dma_start(dbg["v1"][:], y0buf[:].bitcast(F32))

        # =============================================================
        # Phases 4/5: performer FF + gating, then block FFN + residual
        # =============================================================
        with ExitStack() as ph45:
            strm = ph45.enter_context(tc.tile_pool(name="p4s", bufs=2))
            one4 = ph45.enter_context(tc.tile_pool(name="p4o", bufs=1))
            st = ph45.enter_context(tc.tile_pool(name="p4st", bufs=1))
            fbig = ph45.enter_context(tc.tile_pool(name="p4b", bufs=1))
            psums = ph45.enter_context(tc.tile_pool(name="p4ps", bufs=2, space="PSUM"))
            lnpools = (strm, st, psums)

            def ffn_phase(src_fn, g, b, w1_v, b1_t, w2_v, out_cb):
                for t2 in range(2):
                    src = src_fn(t2)
                    y2t = one4.tile([128, DK, 512], MMDT, tag="y2t")
                    layernorm(lambda kk: src(kk),
                              lambda kk: y2t[:, kk, :], 512, g, b, lnpools)
                    h1 = fbig.tile([128, 32, 512], MMDT, tag="h1")
                    for m in range(32):
                        wt = strm.tile([128, DK, 128], MMDT, tag="w1")
                        nc.sync.dma_start(wt[:], w1_v[:, :, ts(m, 128)])
                        ph = psums.tile([128, 512], F32, tag="mm")
                        for kk in range(DK):
                            nc.tensor.matmul(ph[:], r(wt[:, kk, :]), r(y2t[:, kk, :]),
                                             start=(kk == 0), stop=(kk == DK - 1))
                        nc.scalar.activation(h1[:, m, :], ph[:], AF.Gelu,
                                             bias=b1_t[:, m:m + 1], scale=1.0)
                    for mo in range(DK):
                        wt2a = strm.tile([128, 16, 128], MMDT, tag="w2")
                        wt2b = strm.tile([128, 16, 128], MMDT, tag="w2")
                        nc.sync.dma_start(wt2a[:], w2_v[:, 0:16, ts(mo, 128)])
                        nc.sync.dma_start(wt2b[:], w2_v[:, 16:32, ts(mo, 128)])
                        pv = psums.tile([128, 512], F32, tag="mm")
                        for ks in range(32):
                            wt2 = wt2a if ks < 16 else wt2b
                            nc.tensor.matmul(pv[:], r(wt2[:, ks % 16, :]),
                                             r(h1[:, ks, :]),
                                             start=(ks == 0), stop=(ks == 31))
                        out_cb(mo, t2, pv)

            def pff_out(mo, t2, pv):
                t2s = ds(t2 * 512, 512)
                ut = strm.tile([128, 512], F32, tag="ut")
                nc.sync.dma_start(ut[:], u_dram[ts(mo, 128), ds(t2 * 512, 512)])
                xt = strm.tile([128, 512], MMDT, tag="xt")
                nc.sync.dma_start(xt[:], xT_v[:, mo, ds(t2 * 512, 512)])
                v2t = strm.tile([128, 512], F32, tag="v2t")
                nc.vector.scalar_tensor_tensor(v2t[:], pv[:], pb2_t[:, mo:mo + 1],
                                               y0buf[:, mo, t2s], op0=OP.add,
                                               op1=OP.add)
                t3 = strm.tile([128, 512], F32, tag="t3")
                nc.vector.tensor_mul(t3[:], v2t[:], ut[:])
                xo = strm.tile([128, 512], MMDT, tag="ot")
                nc.vector.tensor_add(xo[:], t3[:], xt[:])
                nc.sync.dma_start(x1_dram[ts(mo, 128), ds(t2 * 512, 512)], xo[:])

            ffn_phase(lambda t2: (lambda kk, s=ds(t2 * 512, 512): y0buf[:, kk, s]),
                      flng, flnb, pW1_v, pb1_t, pW2_v, pff_out)

            x1t_ref = {}

            def x1_loader(t2):
                x1t = one4.tile([128, DK, 512], MMDT, tag="x1t")
                nc.sync.dma_start(
                    x1t[:], x1_dram.rearrange("(kk p) t -> p kk t", p=128)
                    [:, :, ds(t2 * 512, 512)])
                x1t_ref["t"] = x1t
                return lambda kk, t=x1t: t[:, kk, :]

            def ffn2_out(mo, t2, pv):
                x1t = x1t_ref["t"]
                ot = strm.tile([128, 512], F32, tag="ot")
                nc.vector.scalar_tensor_tensor(ot[:], pv[:], bf2_t[:, mo:mo + 1],
                                               x1t[:, mo, :], op0=OP.add,
                                               op1=OP.add)
                nc.sync.dma_start(outT[ts(mo, 128), ds(t2 * 512, 512)], ot[:])

            ffn_phase(x1_loader, lng, lnb, Wf1_v, bf1_t, Wf2_v, ffn2_out)

    nc.compile()
    return nc


_NC_CACHE = {}


def _get_nc():
    if "nc" not in _NC_CACHE:
        _NC_CACHE["nc"] = build_nc()
    return _NC_CACHE["nc"]


def make_in_maps(inputs):
    x = np.asarray(inputs["x"], dtype=np.float32)
    projTdn = np.ascontiguousarray(np.asarray(inputs["proj_mat"], np.float32).T * DN)
    common = {k: np.ascontiguousarray(np.asarray(inputs[k], np.float32))
              for k in list(WEIGHT_SHAPES) + list(VEC_SHAPES)}
    common["projTdn"] = projTdn
    in_maps = []
    for c in range(N_CORES):
        b, off = c // 2, (c % 2) * TM
        x_rot = np.roll(x[b], -off, axis=0)            # my tokens first
        m = dict(common)
        m["xT"] = np.ascontiguousarray(x_rot.T)        # [D, TF]
        in_maps.append(m)
    return in_maps


def _run(inputs, trace=False):
    nc = _get_nc()
    in_maps = make_in_maps(inputs)
    res = run_bass_kernel_spmd(nc, in_maps, core_ids=list(range(N_CORES)),
                               trace=trace)
    x = np.asarray(inputs["x"], dtype=np.float32)
    out = np.empty_like(x)
    for c in range(N_CORES):
        b, off = c // 2, (c % 2) * TM
        out[b, off:off + TM] = res.results[c]["outT"].T
    return out, res


def kernel(**inputs):
    out, _ = _run(inputs, trace=False)
    return out
